# revision 37
# baseline (speedup 1.0000x reference)
"""FENet (7-layer stride-2 conv feature extractor) on 8 Trainium2 NeuronCores.

v4 strategy (v3 + overlap/selector fixes)
-----------------------------------------
Two-checkpoint banded composite, weight-stationary:
  stage A: f1 = |feat1*x| (band 40) and [f2|P2] stacked groups (band 118,
           composed to act directly on x); stage B: f3..f7 + final tap from
           the P2 checkpoint.  Data parallel over batch: 8 cores x 3072.
v4 changes over v3:
  - output path: [8, N] feature sums DMA'd straight from SBUF; per-feature
    scale + transpose happen on the host (kills transposes/fc burst).
  - drains split scalar/vector: scalar Abs on cols [0:DS), vector
    cast + sign-bit mask on the rest, halving PSUM-release latency.
  - selector in fp8 DoubleRow for the f1/f2 za tiles (6 tiles -> 3 pair
    matmuls; fp8-safe because those features average 469/254 positions);
    f3..f8 za stays bf16.  fp8 and bf16 parts accumulate in SEPARATE psum
    tiles (mixing modes in one accumulation group corrupts it) and are
    summed on the host.  DoubleRow needs stationary free >= 16 and
    h-half column groups must not interleave.
  - per-chunk x tiles + chunk-major half/quarter DMA pieces so the first
    matmul starts as soon as chunk 0 cols [0:512) land; weights stream on
    the second DMA queue (gpsimd).
"""

import os
import sys

import numpy as np

for _p in ("/opt/trn_rl_repo", os.path.expanduser("~/.axon_site/_ro/trn_rl_repo")):
    if os.path.isdir(_p) and _p not in sys.path:
        sys.path.insert(0, _p)

import concourse.bass as bass
import concourse.bacc as bacc
import concourse.mybir as mybir
from concourse import tile
from concourse.bass_utils import run_bass_kernel_spmd

F32 = mybir.dt.float32
BF16 = mybir.dt.bfloat16
FP8 = mybir.dt.float8e4
NP_BF16 = mybir.dt.np(BF16)
NP_FP8 = mybir.dt.np(FP8)

N_CORES = 8
B_FULL = 24576
L_IN = 900
L_PAD = 1024
B_LOC = B_FULL // N_CORES          # 3072
N_SUB = 512                        # samples per matmul moving tile
T_SUB = 3                          # sub-tiles per pass
N_PASS = 2
NCH_X = 8

KER, STR, PAD_L, PAD_R = 40, 2, 38, 39
N_LAYERS = 7

L1, L2 = 469, 254
NCH_P2 = 2

SRC_CHW = {
    "x":  [128] * 7 + [4],
    "P2": [128, 126],
}
FEAT_SCALE = [1.0 / 469, 1.0 / 254, 1.0 / 146, 1.0 / 92,
              1.0 / 65, 1.0 / 52, 1.0 / 45, 1.0 / 32]


# ----------------------------------------------------------------- host math
def _conv_map(M, w):
    Mp = np.pad(M, ((PAD_L, PAD_R), (0, 0)))
    Lo = (Mp.shape[0] - KER) // STR + 1
    out = np.zeros((Lo, M.shape[1]), dtype=M.dtype)
    for k in range(KER):
        out += w[k] * Mp[k : k + STR * Lo : STR, :]
    return out


def _build_groups(feat_w, pass_w):
    """Device row-groups.  Each group dict:
      src   'x' | 'P2'
      M     [R<=128, C] fp64 rows
      secs  list of (kind, off, rows, meta):
              ('f', row_off, n, (zt, za_off, [(fid, n), ...]))
              ('P', row_off, n, (dst_chunk, dst_off))
    Group order is the per-pass PE issue order.
    """
    fw = feat_w.reshape(N_LAYERS, KER).astype(np.float64)
    pw = pass_w.reshape(N_LAYERS, KER).astype(np.float64)

    I900 = np.eye(L_IN)
    F1 = _conv_map(I900, fw[0])          # [469, 900]
    P1 = _conv_map(I900, pw[0])
    F2 = _conv_map(P1, fw[1])            # [254, 900]  (composed: acts on x)
    P2 = _conv_map(P1, pw[1])            # [254, 900]
    I254 = np.eye(L2)
    F3 = _conv_map(I254, fw[2])
    P3 = _conv_map(I254, pw[2])
    F4 = _conv_map(P3, fw[3])
    P4 = _conv_map(P3, pw[3])
    F5 = _conv_map(P4, fw[4])
    P5 = _conv_map(P4, pw[4])
    F6 = _conv_map(P5, fw[5])
    P6 = _conv_map(P5, pw[5])
    F7 = _conv_map(P6, fw[6])
    PF = _conv_map(P6, pw[6])

    groups = []

    def f1g(a, b, zt):
        n = b - a
        groups.append(dict(src="x", M=F1[a:b], order=None,
                           secs=[("f", 0, n, (zt, 0, [(0, n)]))]))

    def s2g(a, b, zt, za_off, pchunk, poff):
        n = b - a
        # P-half sits at row offset 64 (engine partition bases must be
        # 32-aligned); any gap rows are zero weights -> zero psum rows
        M = np.concatenate([F2[a:b], np.zeros((64 - n, L_IN)), P2[a:b]])
        groups.append(dict(src="x", M=M, order=None,
                           secs=[("f", 0, n, (zt, za_off, [(1, n)])),
                                 ("P", 64, n, (pchunk, poff))]))

    # stage B rows in PSUM order (narrow-support first)
    s3rows = []
    s3rows += [(2, F3[r]) for r in range(64)]
    s3rows += [(3, F4[r]) for r in range(32)]
    s3rows += [(2, F3[r]) for r in range(64, 146)]
    s3rows += [(3, F4[r]) for r in range(32, 92)]
    s3rows += [(4, F5[r]) for r in range(65)]
    s3rows += [(5, F6[r]) for r in range(52)]
    s3rows += [(6, F7[r]) for r in range(45)]
    s3rows += [(7, PF[r]) for r in range(45)]
    assert len(s3rows) == 445
    s3_bounds = [0, 96, 224, 352, 445]

    def s3g(a, b, zt):
        blk = s3rows[a:b]
        M = np.stack([v for _, v in blk])
        feats = []
        for fid, _ in blk:
            if feats and feats[-1][0] == fid:
                feats[-1][1] += 1
            else:
                feats.append([fid, 1])
        groups.append(dict(src="P2", M=M, order=None,
                           secs=[("f", 0, b - a,
                                  (zt, 0, [(f, n) for f, n in feats]))]))

    # issue order: interleave f1/s2 by chunk availability; s3 last with a
    # 2-group gap after the final P2 copy
    f1g(0, 128, 0)
    s2g(0, 64, 4, 0, 0, 0)
    f1g(128, 256, 1)
    s2g(64, 128, 4, 64, 0, 64)
    s2g(128, 192, 5, 0, 1, 0)
    s2g(192, 254, 5, 64, 1, 64)
    f1g(256, 384, 2)
    f1g(384, 469, 3)
    for i, (a, b) in enumerate(zip(s3_bounds[:-1], s3_bounds[1:])):
        s3g(a, b, 6 + i)
    return groups


# zt tile remap: fp8 tiles 0-5 reordered so the DoubleRow pairs (0,1) (2,3)
# (4,5) have matching/compatible row counts; tiles 6-9 stay bf16
ZT_MAP = {0: 0, 1: 1, 2: 2, 4: 3, 3: 4, 5: 5, 6: 6, 7: 7, 8: 8, 9: 9}
N_ZT8 = 6          # fp8 za tiles (features 1-2: large pools, fp8-safe)
N_ZTB = 4          # bf16 za tiles (features 3-8)


def _pack_operands(groups):
    sched = []
    wts = []
    n_zt = 10
    sel = np.zeros((n_zt, 128, 8), dtype=np.float64)
    zt_rows = [0] * n_zt
    for g in groups:
        src, M = g["src"], g["M"]
        mrows = M.shape[0]
        chw = SRC_CHW[src]
        chunks = []
        for c in range(len(chw)):
            sub = M[:, c * 128 : c * 128 + chw[c]]
            if not np.any(sub != 0.0):
                continue
            chunks.append((len(wts), c))
            wts.append((sub.T, chw[c], mrows))
        secs = []
        for kind, off, n, meta in g["secs"]:
            if kind == "f":
                zt, za_off, feats = meta
                zt = ZT_MAP[zt]
                r = za_off
                for fid, cnt in feats:
                    sel[zt, r : r + cnt, fid] = 1.0
                    r += cnt
                zt_rows[zt] = max(zt_rows[zt], za_off + n)
                secs.append((kind, off, n, (zt, za_off, feats)))
            else:
                secs.append((kind, off, n, meta))
        sched.append(dict(src=src, chunks=chunks, mrows=mrows, secs=secs))

    n_wt = len(wts)
    wt = np.zeros((n_wt, 128, 128), dtype=np.float32)
    for i, (m, kw, mrows) in enumerate(wts):
        wt[i, :kw, :mrows] = m
    fscale = np.array(FEAT_SCALE, dtype=np.float32).reshape(8, 1)
    wt = np.ascontiguousarray(wt.transpose(1, 0, 2)).astype(NP_BF16)
    sel_f32 = sel.astype(np.float32).transpose(1, 0, 2)  # [128, zt, 8]
    sel8 = np.zeros((128, N_ZT8, 16), dtype=np.float32)
    sel8[:, :, :8] = sel_f32[:, :N_ZT8, :]
    sel8 = sel8.astype(NP_FP8)
    selb = np.ascontiguousarray(sel_f32[:, N_ZT8:, :]).astype(NP_BF16)
    return wt, sel8, selb, fscale, sched, zt_rows


# ------------------------------------------------- ldweights dedup (post-fin)
def _dedup_ldweights(nc):
    removed = 0
    for blk in nc.main_func.blocks:
        cur = None
        keep = []
        for inst in blk.instructions:
            if isinstance(inst, mybir.InstLdweights):
                sig = (repr(inst.ins[0]), inst.tile_position,
                       inst.perf_mode, inst.is_transpose)
                si = inst.sync_info
                clean = si is None or (len(si.on_wait) == 0
                                       and len(si.on_update) == 0)
                if clean and cur == sig:
                    removed += 1
                    continue
                cur = sig
                keep.append(inst)
            else:
                if isinstance(inst, mybir.InstMatmult):
                    if inst.is_transpose or inst.ldweights is not False:
                        cur = None
                keep.append(inst)
        del blk.instructions[:]
        blk.instructions.extend(keep)
    return removed


# ------------------------------------------------------------ device program
def _build_program(sched, n_wt, zt_rows):
    n_zt = len(zt_rows)
    nc = bacc.Bacc()
    xs_d = nc.dram_tensor("xs", [L_PAD, B_LOC], BF16, kind="ExternalInput")
    wt_d = nc.dram_tensor("wt", [128, n_wt, 128], BF16, kind="ExternalInput")
    sel8_d = nc.dram_tensor("sel8", [128, N_ZT8, 16], FP8,
                            kind="ExternalInput")
    selb_d = nc.dram_tensor("selb", [128, N_ZTB, 8], BF16, kind="ExternalInput")
    out_d = nc.dram_tensor("out", [2, 8, B_LOC], F32,
                           kind="ExternalOutput")

    TT = T_SUB * N_SUB                  # 1536
    QW = B_LOC // 4                     # x DMA piece width (768)
    with tile.TileContext(nc) as tc:
        with (
            tc.tile_pool(name="const", bufs=1) as constp,
            tc.tile_pool(name="ckpt", bufs=1) as ckptp,
            tc.tile_pool(name="za", bufs=2) as zapool,
        ):
            # DMA priority order: weights first (every ldweights needs its
            # slice), then the h=0 half of every x chunk (pass 1 reads only
            # cols [0:TT)), then the h=1 halves.  Two queues (sync, gpsimd)
            # share the load; per-chunk tiles keep deps range-granular.
            sel8_sb = constp.tile([128, N_ZT8, 16], FP8)
            nc.gpsimd.dma_start(sel8_sb[:], sel8_d[:])
            selb_sb = constp.tile([128, N_ZTB, 8], BF16)
            nc.gpsimd.dma_start(selb_sb[:], selb_d[:])
            wt_sb = constp.tile([128, n_wt, 128], BF16)
            wq = (n_wt + 3) // 4
            for w0 in range(0, n_wt, wq):
                w1 = min(n_wt, w0 + wq)
                nc.gpsimd.dma_start(wt_sb[:, w0:w1, :], wt_d[:, w0:w1, :])

            HW_ = B_LOC // 2
            x_cs = [constp.tile([128, B_LOC], BF16, name=f"x_c{c}")
                    for c in range(NCH_X)]
            # chunks 0-1 h0 split into quarters: the very first matmul only
            # needs cols [0:512) of chunk 0, so its wait clears ~1.5us sooner
            for c in range(2):
                for q0, q1 in ((0, 512), (512, 1024), (1024, 1536)):
                    nc.sync.dma_start(
                        x_cs[c][:, q0:q1],
                        xs_d[c * 128 : (c + 1) * 128, q0:q1])
            for h in range(2):
                for c in range(NCH_X):
                    if h == 0 and c < 2:
                        continue
                    nc.sync.dma_start(
                        x_cs[c][:, h * HW_ : (h + 1) * HW_],
                        xs_d[c * 128 : (c + 1) * 128, h * HW_ : (h + 1) * HW_])

            p2_cs = [ckptp.tile([128, B_LOC], BF16, name=f"p2_c{c}")
                     for c in range(NCH_P2)]

            # split point for drain work: scalar takes [0:DS), vector the rest
            # (scalar: 0.833ns/el Abs; vector: 1.042 cast + 0.26 sign-mask)
            DS = 960

            with (
                tc.tile_pool(name="pg", bufs=2,
                             space=bass.MemorySpace.PSUM) as pgp,
                tc.tile_pool(name="pf", bufs=1,
                             space=bass.MemorySpace.PSUM) as pfp,
            ):
                for p in range(N_PASS):
                    s0 = p * TT
                    za8 = zapool.tile([128, N_ZT8, TT], FP8, tag="za8")
                    zab = zapool.tile([128, N_ZTB, TT], BF16, tag="zab")
                    # fp8 pair (4,5) contracts K=126 but tile 4 only has 85
                    # real rows: zero the tail so the PE never reads garbage
                    nc.vector.memset(za8[64:128, 4, :], 0.0)
                    for ent in sched:
                        src, mrows = ent["src"], ent["mrows"]
                        src_cs = x_cs if src == "x" else p2_cs
                        pg = pgp.tile([128, T_SUB, N_SUB], F32, tag="pg")
                        nj = len(ent["chunks"])
                        for j, (i, c) in enumerate(ent["chunks"]):
                            kw = SRC_CHW[src][c]
                            for t in range(T_SUB):
                                nc.tensor.matmul(
                                    pg[0:mrows, t, :],
                                    wt_sb[0:kw, i, 0:mrows],
                                    src_cs[c][0:kw,
                                              s0 + t * N_SUB : s0 + (t + 1) * N_SUB],
                                    start=(j == 0), stop=(j == nj - 1),
                                    skip_group_check=True)
                        for kind, off, n, meta in ent["secs"]:
                            view = pg[off : off + n].rearrange(
                                "p t n -> p (t n)")
                            if kind == "f":
                                zt, za_off, _ = meta
                                if zt < N_ZT8:
                                    dst = za8[za_off : za_off + n, zt, :]
                                    idt, msk = mybir.dt.uint8, 0x7F
                                else:
                                    dst = zab[za_off : za_off + n,
                                              zt - N_ZT8, :]
                                    idt, msk = mybir.dt.uint16, 0x7FFF
                                nc.scalar.activation(
                                    dst[:, 0:DS], view[:, 0:DS],
                                    mybir.ActivationFunctionType.Abs)
                                nc.vector.tensor_copy(
                                    dst[:, DS:TT], view[:, DS:TT])
                                du = dst[:, DS:TT].bitcast(idt)
                                nc.vector.tensor_scalar(
                                    du, du, msk, None,
                                    op0=mybir.AluOpType.bitwise_and)
                            else:
                                dc, doff = meta
                                dst = p2_cs[dc][doff : doff + n,
                                               s0 : s0 + TT]
                                nc.scalar.copy(dst[:, 0:DS], view[:, 0:DS])
                                nc.vector.tensor_copy(
                                    dst[:, DS:TT], view[:, DS:TT])

                    # selector burst + output for this pass: accumulate
                    # [8, N] feature sums in PSUM, copy to SBUF, DMA out
                    # (scale + transpose happen on the host)
                    # fp8 DoubleRow pairs and bf16 singles must NOT share
                    # one PSUM accumulation group (mode switch corrupts it):
                    # separate tiles, combined on the vector engine
                    HN = N_SUB // 2
                    NP8 = N_ZT8 // 2
                    for t in range(T_SUB):
                        pf8 = pfp.tile([16, N_SUB], F32, tag="pf8")
                        pfb = pfp.tile([8, N_SUB], F32, tag="pfb")
                        for h in range(2):
                            for pi in range(NP8):
                                kr = max(zt_rows[2 * pi], zt_rows[2 * pi + 1])
                                c0 = t * N_SUB + h * HN
                                nc.tensor.matmul(
                                    pf8[0:16, h * HN : (h + 1) * HN],
                                    sel8_sb[0:kr, 2 * pi : 2 * pi + 2, :],
                                    za8[0:kr, 2 * pi : 2 * pi + 2,
                                        c0 : c0 + HN],
                                    start=(pi == 0), stop=(pi == NP8 - 1),
                                    perf_mode=mybir.MatmulPerfMode.DoubleRow,
                                    skip_group_check=True)
                        for j in range(N_ZTB):
                            kr = zt_rows[N_ZT8 + j]
                            nc.tensor.matmul(
                                pfb[:],
                                selb_sb[0:kr, j, :],
                                zab[0:kr, j, t * N_SUB : (t + 1) * N_SUB],
                                start=(j == 0), stop=(j == N_ZTB - 1),
                                skip_group_check=True)
                        trow = s0 + t * N_SUB
                        fc8 = zapool.tile([8, N_SUB], F32, tag="fc8")
                        nc.vector.tensor_copy(fc8[:], pf8[0:8, :])
                        fcb = zapool.tile([8, N_SUB], F32, tag="fcb")
                        nc.scalar.copy(fcb[:], pfb[:])
                        nc.gpsimd.dma_start(
                            out_d[0, :, trow : trow + N_SUB], fc8[:])
                        nc.gpsimd.dma_start(
                            out_d[1, :, trow : trow + N_SUB], fcb[:])
    nc.finalize()
    _dedup_ldweights(nc)
    return nc


_CACHE = {}


def _get_program(feat_w, pass_w):
    groups = _build_groups(feat_w, pass_w)
    wt, sel8, selb, fscale, sched, zt_rows = _pack_operands(groups)
    key = tuple((e["src"], e["mrows"], tuple(e["chunks"]), repr(e["secs"]))
                for e in sched)
    if key not in _CACHE:
        _CACHE[key] = _build_program(sched, wt.shape[1], zt_rows)
    return _CACHE[key], wt, sel8, selb, fscale


def _prepare(inputs):
    nc, wt, sel8, selb, fscale = _get_program(
        inputs["feat_w"], inputs["pass_w"])
    xsT = np.zeros((L_PAD, B_FULL), dtype=NP_BF16)
    xsT[:L_IN, :] = np.asarray(
        inputs["x"], dtype=np.float32).reshape(B_FULL, L_IN).T
    in_maps = [
        {"xs": np.ascontiguousarray(xsT[:, i * B_LOC : (i + 1) * B_LOC]),
         "wt": wt, "sel8": sel8, "selb": selb}
        for i in range(N_CORES)
    ]

    def post(res):
        out = np.concatenate(
            [res.results[i]["out"].sum(axis=0) for i in range(N_CORES)],
            axis=1)
        return np.ascontiguousarray((out * fscale).T.astype(np.float32))

    return nc, in_maps, post


def kernel(x, feat_w, pass_w):
    nc, in_maps, post = _prepare(
        {"x": x, "feat_w": feat_w, "pass_w": pass_w})
    res = run_bass_kernel_spmd(nc, in_maps, list(range(N_CORES)))
    return post(res)


if __name__ == "__main__":
    rng = np.random.default_rng(0)
    feat_w = (rng.standard_normal((7, 1, 1, 40)) * 0.1).astype(np.float32)
    pass_w = (rng.standard_normal((7, 1, 1, 40)) * 0.1).astype(np.float32)
    groups = _build_groups(feat_w, pass_w)
    wt, sel8, selb, fscale, sched, zt_rows = _pack_operands(groups)
    sel = np.concatenate([sel8.astype(np.float32),
                          selb.astype(np.float32)], axis=1)
    n_mm = sum(len(e["chunks"]) for e in sched)
    print(f"groups={len(sched)} mms/subtile={n_mm} zt_rows={zt_rows}")
    for e in sched:
        print(f"  {e['src']:3s} rows={e['mrows']:3d} "
              f"chunks={[c for _, c in e['chunks']]} "
              f"secs={[(k, o, n) for k, o, n, _ in e['secs']]}")

    def bf(a):
        return np.asarray(a, dtype=np.float32).astype(NP_BF16).astype(np.float64)

    B = 256
    x = rng.standard_normal((B, 1, L_IN)).astype(np.float32)
    xs = np.zeros((L_PAD, B))
    xs[:L_IN] = bf(x.reshape(B, L_IN).T)
    srcs = {"x": xs, "P2": np.zeros((NCH_P2 * 128, B))}
    za = np.zeros((10, 128, B))
    wtf = np.ascontiguousarray(wt.transpose(1, 0, 2)).astype(np.float64)
    for e in sched:
        acc = np.zeros((128, B))
        S = srcs[e["src"]]
        for i, c in e["chunks"]:
            kw = SRC_CHW[e["src"]][c]
            acc[: e["mrows"]] += (
                wtf[i, :kw, : e["mrows"]].T @ S[c * 128 : c * 128 + kw])
        for kind, off, n, meta in e["secs"]:
            if kind == "f":
                zt, za_off, _ = meta
                q = np.abs(acc[off : off + n]).astype(np.float32)
                if zt < N_ZT8:
                    q = q.astype(NP_FP8).astype(np.float64)
                else:
                    q = bf(q)
                za[zt, za_off : za_off + n] = q
            else:
                dc, doff = meta
                srcs["P2"][dc * 128 + doff : dc * 128 + doff + n] = bf(
                    acc[off : off + n])
    self_sel = np.ascontiguousarray(sel.transpose(1, 0, 2)).astype(np.float64)
    pf = np.zeros((8, B))
    for zt in range(10):
        kr = zt_rows[zt]
        pf += self_sel[zt, :kr].T @ za[zt, :kr]
    feats = (pf * fscale.astype(np.float64)).T

    fw = feat_w.reshape(7, 40).astype(np.float64)
    pw = pass_w.reshape(7, 40).astype(np.float64)
    px = x.reshape(B, L_IN).astype(np.float64)
    ref = []
    cur = px
    for i in range(7):
        xp = np.pad(cur, ((0, 0), (PAD_L, PAD_R)))
        Lo = (xp.shape[1] - KER) // STR + 1
        f = np.zeros((B, Lo)); nxt = np.zeros((B, Lo))
        for k in range(KER):
            sl = xp[:, k : k + STR * Lo : STR]
            f += fw[i, k] * sl
            nxt += pw[i, k] * sl
        ref.append(np.abs(f).mean(1))
        cur = nxt
    ref.append(np.abs(cur).sum(1) / 32.0)
    ref = np.stack(ref, 1)
    rel = np.abs(feats - ref) / np.maximum(np.abs(ref), 1e-6)
    print(f"host-emulated rel err vs fp64 reference: {rel.max():.3e}")



# revision 38
# speedup vs baseline: 1.1082x; 1.1082x over previous
"""FENet (7-layer stride-2 conv feature extractor) on 8 Trainium2 NeuronCores.

v4 strategy (v3 + overlap/selector fixes)
-----------------------------------------
Two-checkpoint banded composite, weight-stationary:
  stage A: f1 = |feat1*x| (band 40) and [f2|P2] stacked groups (band 118,
           composed to act directly on x); stage B: f3..f7 + final tap from
           the P2 checkpoint.  Data parallel over batch: 8 cores x 3072.
v4 changes over v3:
  - output path: [8, N] feature sums DMA'd straight from SBUF; per-feature
    scale + transpose happen on the host (kills transposes/fc burst).
  - drains split scalar/vector: scalar Abs on cols [0:DS), vector
    cast + sign-bit mask on the rest, halving PSUM-release latency.
  - selector in fp8 DoubleRow for the f1/f2 za tiles (6 tiles -> 3 pair
    matmuls; fp8-safe because those features average 469/254 positions);
    f3..f8 za stays bf16.  fp8 and bf16 parts accumulate in SEPARATE psum
    tiles (mixing modes in one accumulation group corrupts it) and are
    summed on the host.  DoubleRow needs stationary free >= 16 and
    h-half column groups must not interleave.
  - per-chunk x tiles + chunk-major half/quarter DMA pieces so the first
    matmul starts as soon as chunk 0 cols [0:512) land; weights stream on
    the second DMA queue (gpsimd).
"""

import os
import sys

import numpy as np

for _p in ("/opt/trn_rl_repo", os.path.expanduser("~/.axon_site/_ro/trn_rl_repo")):
    if os.path.isdir(_p) and _p not in sys.path:
        sys.path.insert(0, _p)

import concourse.bass as bass
import concourse.bacc as bacc
import concourse.mybir as mybir
from concourse import tile
from concourse.bass_utils import run_bass_kernel_spmd

F32 = mybir.dt.float32
BF16 = mybir.dt.bfloat16
FP8 = mybir.dt.float8e4
NP_BF16 = mybir.dt.np(BF16)
NP_FP8 = mybir.dt.np(FP8)

N_CORES = 8
B_FULL = 24576
L_IN = 900
L_PAD = 1024
B_LOC = B_FULL // N_CORES          # 3072
N_SUB = 512                        # samples per matmul moving tile
T_SUB = 2                          # sub-tiles per pass
N_PASS = 3
NCH_X = 8

KER, STR, PAD_L, PAD_R = 40, 2, 38, 39
N_LAYERS = 7

L1, L2 = 469, 254
NCH_P2 = 2

SRC_CHW = {
    "x":  [128] * 7 + [4],
    "P2": [128, 126],
}
FEAT_SCALE = [1.0 / 469, 1.0 / 254, 1.0 / 146, 1.0 / 92,
              1.0 / 65, 1.0 / 52, 1.0 / 45, 1.0 / 32]


# ----------------------------------------------------------------- host math
def _conv_map(M, w):
    Mp = np.pad(M, ((PAD_L, PAD_R), (0, 0)))
    Lo = (Mp.shape[0] - KER) // STR + 1
    out = np.zeros((Lo, M.shape[1]), dtype=M.dtype)
    for k in range(KER):
        out += w[k] * Mp[k : k + STR * Lo : STR, :]
    return out


def _build_groups(feat_w, pass_w):
    """Device row-groups.  Each group dict:
      src   'x' | 'P2'
      M     [R<=128, C] fp64 rows
      secs  list of (kind, off, rows, meta):
              ('f', row_off, n, (zt, za_off, [(fid, n), ...]))
              ('P', row_off, n, (dst_chunk, dst_off))
    Group order is the per-pass PE issue order.
    """
    fw = feat_w.reshape(N_LAYERS, KER).astype(np.float64)
    pw = pass_w.reshape(N_LAYERS, KER).astype(np.float64)

    I900 = np.eye(L_IN)
    F1 = _conv_map(I900, fw[0])          # [469, 900]
    P1 = _conv_map(I900, pw[0])
    F2 = _conv_map(P1, fw[1])            # [254, 900]  (composed: acts on x)
    P2 = _conv_map(P1, pw[1])            # [254, 900]
    I254 = np.eye(L2)
    F3 = _conv_map(I254, fw[2])
    P3 = _conv_map(I254, pw[2])
    F4 = _conv_map(P3, fw[3])
    P4 = _conv_map(P3, pw[3])
    F5 = _conv_map(P4, fw[4])
    P5 = _conv_map(P4, pw[4])
    F6 = _conv_map(P5, fw[5])
    P6 = _conv_map(P5, pw[5])
    F7 = _conv_map(P6, fw[6])
    PF = _conv_map(P6, pw[6])

    groups = []

    def f1g(a, b, zt):
        n = b - a
        groups.append(dict(src="x", M=F1[a:b], order=None,
                           secs=[("f", 0, n, (zt, 0, [(0, n)]))]))

    def s2g(a, b, zt, za_off, pchunk, poff):
        n = b - a
        # P-half sits at row offset 64 (engine partition bases must be
        # 32-aligned); any gap rows are zero weights -> zero psum rows
        M = np.concatenate([F2[a:b], np.zeros((64 - n, L_IN)), P2[a:b]])
        groups.append(dict(src="x", M=M, order=None,
                           secs=[("f", 0, n, (zt, za_off, [(1, n)])),
                                 ("P", 64, n, (pchunk, poff))]))

    # stage B rows in PSUM order (narrow-support first)
    s3rows = []
    s3rows += [(2, F3[r]) for r in range(64)]
    s3rows += [(3, F4[r]) for r in range(32)]
    s3rows += [(2, F3[r]) for r in range(64, 146)]
    s3rows += [(3, F4[r]) for r in range(32, 92)]
    s3rows += [(4, F5[r]) for r in range(65)]
    s3rows += [(5, F6[r]) for r in range(52)]
    s3rows += [(6, F7[r]) for r in range(45)]
    s3rows += [(7, PF[r]) for r in range(45)]
    assert len(s3rows) == 445
    s3_bounds = [0, 96, 224, 352, 445]

    def s3g(a, b, zt):
        blk = s3rows[a:b]
        M = np.stack([v for _, v in blk])
        feats = []
        for fid, _ in blk:
            if feats and feats[-1][0] == fid:
                feats[-1][1] += 1
            else:
                feats.append([fid, 1])
        groups.append(dict(src="P2", M=M, order=None,
                           secs=[("f", 0, b - a,
                                  (zt, 0, [(f, n) for f, n in feats]))]))

    # issue order: interleave f1/s2 by chunk availability; s3 last with a
    # 2-group gap after the final P2 copy
    f1g(0, 128, 0)
    s2g(0, 64, 4, 0, 0, 0)
    f1g(128, 256, 1)
    s2g(64, 128, 4, 64, 0, 64)
    s2g(128, 192, 5, 0, 1, 0)
    s2g(192, 254, 5, 64, 1, 64)
    f1g(256, 384, 2)
    f1g(384, 469, 3)
    for i, (a, b) in enumerate(zip(s3_bounds[:-1], s3_bounds[1:])):
        s3g(a, b, 6 + i)
    return groups


# zt tile remap: fp8 tiles 0-5 reordered so the DoubleRow pairs (0,1) (2,3)
# (4,5) have matching/compatible row counts; tiles 6-9 stay bf16
ZT_MAP = {0: 0, 1: 1, 2: 2, 4: 3, 3: 4, 5: 5, 6: 6, 7: 7, 8: 8, 9: 9}
N_ZT8 = 6          # fp8 za tiles (features 1-2: large pools, fp8-safe)
N_ZTB = 4          # bf16 za tiles (features 3-8)


def _pack_operands(groups):
    sched = []
    wts = []
    n_zt = 10
    sel = np.zeros((n_zt, 128, 8), dtype=np.float64)
    zt_rows = [0] * n_zt
    for g in groups:
        src, M = g["src"], g["M"]
        mrows = M.shape[0]
        chw = SRC_CHW[src]
        chunks = []
        for c in range(len(chw)):
            sub = M[:, c * 128 : c * 128 + chw[c]]
            if not np.any(sub != 0.0):
                continue
            chunks.append((len(wts), c))
            wts.append((sub.T, chw[c], mrows))
        secs = []
        for kind, off, n, meta in g["secs"]:
            if kind == "f":
                zt, za_off, feats = meta
                zt = ZT_MAP[zt]
                r = za_off
                for fid, cnt in feats:
                    sel[zt, r : r + cnt, fid] = 1.0
                    r += cnt
                zt_rows[zt] = max(zt_rows[zt], za_off + n)
                secs.append((kind, off, n, (zt, za_off, feats)))
            else:
                secs.append((kind, off, n, meta))
        sched.append(dict(src=src, chunks=chunks, mrows=mrows, secs=secs))

    n_wt = len(wts)
    wt = np.zeros((n_wt, 128, 128), dtype=np.float32)
    for i, (m, kw, mrows) in enumerate(wts):
        wt[i, :kw, :mrows] = m
    fscale = np.array(FEAT_SCALE, dtype=np.float32).reshape(8, 1)
    wt = np.ascontiguousarray(wt.transpose(1, 0, 2)).astype(NP_BF16)
    sel_f32 = sel.astype(np.float32).transpose(1, 0, 2)  # [128, zt, 8]
    sel8 = np.zeros((128, N_ZT8, 16), dtype=np.float32)
    sel8[:, :, :8] = sel_f32[:, :N_ZT8, :]
    sel8 = sel8.astype(NP_FP8)
    selb = np.ascontiguousarray(sel_f32[:, N_ZT8:, :]).astype(NP_BF16)
    return wt, sel8, selb, fscale, sched, zt_rows


# ------------------------------------------------- ldweights dedup (post-fin)
def _dedup_ldweights(nc):
    removed = 0
    for blk in nc.main_func.blocks:
        cur = None
        keep = []
        for inst in blk.instructions:
            if isinstance(inst, mybir.InstLdweights):
                sig = (repr(inst.ins[0]), inst.tile_position,
                       inst.perf_mode, inst.is_transpose)
                si = inst.sync_info
                clean = si is None or (len(si.on_wait) == 0
                                       and len(si.on_update) == 0)
                if clean and cur == sig:
                    removed += 1
                    continue
                cur = sig
                keep.append(inst)
            else:
                if isinstance(inst, mybir.InstMatmult):
                    if inst.is_transpose or inst.ldweights is not False:
                        cur = None
                keep.append(inst)
        del blk.instructions[:]
        blk.instructions.extend(keep)
    return removed


# ------------------------------------------------------------ device program
def _build_program(sched, n_wt, zt_rows):
    n_zt = len(zt_rows)
    nc = bacc.Bacc()
    xs_d = nc.dram_tensor("xs", [L_PAD, B_LOC], BF16, kind="ExternalInput")
    wt_d = nc.dram_tensor("wt", [128, n_wt, 128], BF16, kind="ExternalInput")
    sel8_d = nc.dram_tensor("sel8", [128, N_ZT8, 16], FP8,
                            kind="ExternalInput")
    selb_d = nc.dram_tensor("selb", [128, N_ZTB, 8], BF16, kind="ExternalInput")
    out_d = nc.dram_tensor("out", [2, 8, B_LOC], F32,
                           kind="ExternalOutput")

    TT = T_SUB * N_SUB                  # 1536
    QW = B_LOC // 4                     # x DMA piece width (768)
    with tile.TileContext(nc) as tc:
        with (
            tc.tile_pool(name="const", bufs=1) as constp,
            tc.tile_pool(name="ckpt", bufs=1) as ckptp,
            tc.tile_pool(name="za", bufs=2) as zapool,
        ):
            # DMA priority order: weights first (every ldweights needs its
            # slice), then the h=0 half of every x chunk (pass 1 reads only
            # cols [0:TT)), then the h=1 halves.  Two queues (sync, gpsimd)
            # share the load; per-chunk tiles keep deps range-granular.
            sel8_sb = constp.tile([128, N_ZT8, 16], FP8)
            nc.gpsimd.dma_start(sel8_sb[:], sel8_d[:])
            selb_sb = constp.tile([128, N_ZTB, 8], BF16)
            nc.gpsimd.dma_start(selb_sb[:], selb_d[:])
            wt_sb = constp.tile([128, n_wt, 128], BF16)
            wq = (n_wt + 3) // 4
            for w0 in range(0, n_wt, wq):
                w1 = min(n_wt, w0 + wq)
                nc.gpsimd.dma_start(wt_sb[:, w0:w1, :], wt_d[:, w0:w1, :])

            HW_ = B_LOC // 2
            x_cs = [constp.tile([128, B_LOC], BF16, name=f"x_c{c}")
                    for c in range(NCH_X)]
            # chunks 0-1 h0 split into quarters: the very first matmul only
            # needs cols [0:512) of chunk 0, so its wait clears ~1.5us sooner
            for c in range(2):
                for q0, q1 in ((0, 512), (512, 1024), (1024, 1536)):
                    nc.sync.dma_start(
                        x_cs[c][:, q0:q1],
                        xs_d[c * 128 : (c + 1) * 128, q0:q1])
            for h in range(2):
                for c in range(NCH_X):
                    if h == 0 and c < 2:
                        continue
                    nc.sync.dma_start(
                        x_cs[c][:, h * HW_ : (h + 1) * HW_],
                        xs_d[c * 128 : (c + 1) * 128, h * HW_ : (h + 1) * HW_])

            p2_cs = [ckptp.tile([128, B_LOC], BF16, name=f"p2_c{c}")
                     for c in range(NCH_P2)]

            # split point for drain work: scalar takes [0:DS), vector the rest
            # (scalar: 0.833ns/el Abs; vector: 1.042 cast + 0.26 sign-mask)
            DS = 640

            with (
                tc.tile_pool(name="pg", bufs=3,
                             space=bass.MemorySpace.PSUM) as pgp,
                tc.tile_pool(name="pf", bufs=1,
                             space=bass.MemorySpace.PSUM) as pfp,
            ):
                for p in range(N_PASS):
                    s0 = p * TT
                    za8 = zapool.tile([128, N_ZT8, TT], FP8, tag="za8")
                    zab = zapool.tile([128, N_ZTB, TT], BF16, tag="zab")
                    # fp8 pair (4,5) contracts K=126 but tile 4 only has 85
                    # real rows: zero the tail so the PE never reads garbage
                    nc.vector.memset(za8[64:128, 4, :], 0.0)
                    for ent in sched:
                        src, mrows = ent["src"], ent["mrows"]
                        src_cs = x_cs if src == "x" else p2_cs
                        pg = pgp.tile([128, T_SUB, N_SUB], F32, tag="pg")
                        nj = len(ent["chunks"])
                        for j, (i, c) in enumerate(ent["chunks"]):
                            kw = SRC_CHW[src][c]
                            for t in range(T_SUB):
                                nc.tensor.matmul(
                                    pg[0:mrows, t, :],
                                    wt_sb[0:kw, i, 0:mrows],
                                    src_cs[c][0:kw,
                                              s0 + t * N_SUB : s0 + (t + 1) * N_SUB],
                                    start=(j == 0), stop=(j == nj - 1),
                                    skip_group_check=True)
                        for kind, off, n, meta in ent["secs"]:
                            view = pg[off : off + n].rearrange(
                                "p t n -> p (t n)")
                            if kind == "f":
                                zt, za_off, _ = meta
                                if zt < N_ZT8:
                                    dst = za8[za_off : za_off + n, zt, :]
                                    idt, msk = mybir.dt.uint8, 0x7F
                                else:
                                    dst = zab[za_off : za_off + n,
                                              zt - N_ZT8, :]
                                    idt, msk = mybir.dt.uint16, 0x7FFF
                                nc.scalar.activation(
                                    dst[:, 0:DS], view[:, 0:DS],
                                    mybir.ActivationFunctionType.Abs)
                                nc.vector.tensor_copy(
                                    dst[:, DS:TT], view[:, DS:TT])
                                du = dst[:, DS:TT].bitcast(idt)
                                nc.vector.tensor_scalar(
                                    du, du, msk, None,
                                    op0=mybir.AluOpType.bitwise_and)
                            else:
                                dc, doff = meta
                                dst = p2_cs[dc][doff : doff + n,
                                               s0 : s0 + TT]
                                nc.scalar.copy(dst[:, 0:DS], view[:, 0:DS])
                                nc.vector.tensor_copy(
                                    dst[:, DS:TT], view[:, DS:TT])

                    # selector burst + output for this pass: accumulate
                    # [8, N] feature sums in PSUM, copy to SBUF, DMA out
                    # (scale + transpose happen on the host)
                    # fp8 DoubleRow pairs and bf16 singles must NOT share
                    # one PSUM accumulation group (mode switch corrupts it):
                    # separate tiles, combined on the vector engine
                    HN = N_SUB // 2
                    NP8 = N_ZT8 // 2
                    for t in range(T_SUB):
                        pf8 = pfp.tile([16, N_SUB], F32, tag="pf8")
                        pfb = pfp.tile([8, N_SUB], F32, tag="pfb")
                        for h in range(2):
                            for pi in range(NP8):
                                kr = max(zt_rows[2 * pi], zt_rows[2 * pi + 1])
                                c0 = t * N_SUB + h * HN
                                nc.tensor.matmul(
                                    pf8[0:16, h * HN : (h + 1) * HN],
                                    sel8_sb[0:kr, 2 * pi : 2 * pi + 2, :],
                                    za8[0:kr, 2 * pi : 2 * pi + 2,
                                        c0 : c0 + HN],
                                    start=(pi == 0), stop=(pi == NP8 - 1),
                                    perf_mode=mybir.MatmulPerfMode.DoubleRow,
                                    skip_group_check=True)
                        for j in range(N_ZTB):
                            kr = zt_rows[N_ZT8 + j]
                            nc.tensor.matmul(
                                pfb[:],
                                selb_sb[0:kr, j, :],
                                zab[0:kr, j, t * N_SUB : (t + 1) * N_SUB],
                                start=(j == 0), stop=(j == N_ZTB - 1),
                                skip_group_check=True)
                        trow = s0 + t * N_SUB
                        fc8 = zapool.tile([8, N_SUB], F32, tag="fc8")
                        nc.vector.tensor_copy(fc8[:], pf8[0:8, :])
                        fcb = zapool.tile([8, N_SUB], F32, tag="fcb")
                        nc.scalar.copy(fcb[:], pfb[:])
                        nc.gpsimd.dma_start(
                            out_d[0, :, trow : trow + N_SUB], fc8[:])
                        nc.gpsimd.dma_start(
                            out_d[1, :, trow : trow + N_SUB], fcb[:])
    nc.finalize()
    _dedup_ldweights(nc)
    return nc


_CACHE = {}


def _get_program(feat_w, pass_w):
    groups = _build_groups(feat_w, pass_w)
    wt, sel8, selb, fscale, sched, zt_rows = _pack_operands(groups)
    key = tuple((e["src"], e["mrows"], tuple(e["chunks"]), repr(e["secs"]))
                for e in sched)
    if key not in _CACHE:
        _CACHE[key] = _build_program(sched, wt.shape[1], zt_rows)
    return _CACHE[key], wt, sel8, selb, fscale


def _prepare(inputs):
    nc, wt, sel8, selb, fscale = _get_program(
        inputs["feat_w"], inputs["pass_w"])
    xsT = np.zeros((L_PAD, B_FULL), dtype=NP_BF16)
    xsT[:L_IN, :] = np.asarray(
        inputs["x"], dtype=np.float32).reshape(B_FULL, L_IN).T
    in_maps = [
        {"xs": np.ascontiguousarray(xsT[:, i * B_LOC : (i + 1) * B_LOC]),
         "wt": wt, "sel8": sel8, "selb": selb}
        for i in range(N_CORES)
    ]

    def post(res):
        out = np.concatenate(
            [res.results[i]["out"].sum(axis=0) for i in range(N_CORES)],
            axis=1)
        return np.ascontiguousarray((out * fscale).T.astype(np.float32))

    return nc, in_maps, post


def kernel(x, feat_w, pass_w):
    nc, in_maps, post = _prepare(
        {"x": x, "feat_w": feat_w, "pass_w": pass_w})
    res = run_bass_kernel_spmd(nc, in_maps, list(range(N_CORES)))
    return post(res)


if __name__ == "__main__":
    rng = np.random.default_rng(0)
    feat_w = (rng.standard_normal((7, 1, 1, 40)) * 0.1).astype(np.float32)
    pass_w = (rng.standard_normal((7, 1, 1, 40)) * 0.1).astype(np.float32)
    groups = _build_groups(feat_w, pass_w)
    wt, sel8, selb, fscale, sched, zt_rows = _pack_operands(groups)
    sel = np.concatenate([sel8.astype(np.float32),
                          selb.astype(np.float32)], axis=1)
    n_mm = sum(len(e["chunks"]) for e in sched)
    print(f"groups={len(sched)} mms/subtile={n_mm} zt_rows={zt_rows}")
    for e in sched:
        print(f"  {e['src']:3s} rows={e['mrows']:3d} "
              f"chunks={[c for _, c in e['chunks']]} "
              f"secs={[(k, o, n) for k, o, n, _ in e['secs']]}")

    def bf(a):
        return np.asarray(a, dtype=np.float32).astype(NP_BF16).astype(np.float64)

    B = 256
    x = rng.standard_normal((B, 1, L_IN)).astype(np.float32)
    xs = np.zeros((L_PAD, B))
    xs[:L_IN] = bf(x.reshape(B, L_IN).T)
    srcs = {"x": xs, "P2": np.zeros((NCH_P2 * 128, B))}
    za = np.zeros((10, 128, B))
    wtf = np.ascontiguousarray(wt.transpose(1, 0, 2)).astype(np.float64)
    for e in sched:
        acc = np.zeros((128, B))
        S = srcs[e["src"]]
        for i, c in e["chunks"]:
            kw = SRC_CHW[e["src"]][c]
            acc[: e["mrows"]] += (
                wtf[i, :kw, : e["mrows"]].T @ S[c * 128 : c * 128 + kw])
        for kind, off, n, meta in e["secs"]:
            if kind == "f":
                zt, za_off, _ = meta
                q = np.abs(acc[off : off + n]).astype(np.float32)
                if zt < N_ZT8:
                    q = q.astype(NP_FP8).astype(np.float64)
                else:
                    q = bf(q)
                za[zt, za_off : za_off + n] = q
            else:
                dc, doff = meta
                srcs["P2"][dc * 128 + doff : dc * 128 + doff + n] = bf(
                    acc[off : off + n])
    self_sel = np.ascontiguousarray(sel.transpose(1, 0, 2)).astype(np.float64)
    pf = np.zeros((8, B))
    for zt in range(10):
        kr = zt_rows[zt]
        pf += self_sel[zt, :kr].T @ za[zt, :kr]
    feats = (pf * fscale.astype(np.float64)).T

    fw = feat_w.reshape(7, 40).astype(np.float64)
    pw = pass_w.reshape(7, 40).astype(np.float64)
    px = x.reshape(B, L_IN).astype(np.float64)
    ref = []
    cur = px
    for i in range(7):
        xp = np.pad(cur, ((0, 0), (PAD_L, PAD_R)))
        Lo = (xp.shape[1] - KER) // STR + 1
        f = np.zeros((B, Lo)); nxt = np.zeros((B, Lo))
        for k in range(KER):
            sl = xp[:, k : k + STR * Lo : STR]
            f += fw[i, k] * sl
            nxt += pw[i, k] * sl
        ref.append(np.abs(f).mean(1))
        cur = nxt
    ref.append(np.abs(cur).sum(1) / 32.0)
    ref = np.stack(ref, 1)
    rel = np.abs(feats - ref) / np.maximum(np.abs(ref), 1e-6)
    print(f"host-emulated rel err vs fp64 reference: {rel.max():.3e}")



# revision 40
# speedup vs baseline: 1.1257x; 1.0157x over previous
"""FENet (7-layer stride-2 conv feature extractor) on 8 Trainium2 NeuronCores.

v4 strategy (v3 + overlap/selector fixes)
-----------------------------------------
Two-checkpoint banded composite, weight-stationary:
  stage A: f1 = |feat1*x| (band 40) and [f2|P2] stacked groups (band 118,
           composed to act directly on x); stage B: f3..f7 + final tap from
           the P2 checkpoint.  Data parallel over batch: 8 cores x 3072.
v4 changes over v3:
  - output path: [8, N] feature sums DMA'd straight from SBUF; per-feature
    scale + transpose happen on the host (kills transposes/fc burst).
  - drains split scalar/vector: scalar Abs on cols [0:DS), vector
    cast + sign-bit mask on the rest, halving PSUM-release latency.
  - selector in fp8 DoubleRow for the f1/f2 za tiles (6 tiles -> 3 pair
    matmuls; fp8-safe because those features average 469/254 positions);
    f3..f8 za stays bf16.  fp8 and bf16 parts accumulate in SEPARATE psum
    tiles (mixing modes in one accumulation group corrupts it) and are
    summed on the host.  DoubleRow needs stationary free >= 16 and
    h-half column groups must not interleave.
  - per-chunk x tiles + chunk-major half/quarter DMA pieces so the first
    matmul starts as soon as chunk 0 cols [0:512) land; weights stream on
    the second DMA queue (gpsimd).
"""

import os
import sys

import numpy as np

for _p in ("/opt/trn_rl_repo", os.path.expanduser("~/.axon_site/_ro/trn_rl_repo")):
    if os.path.isdir(_p) and _p not in sys.path:
        sys.path.insert(0, _p)

import concourse.bass as bass
import concourse.bacc as bacc
import concourse.mybir as mybir
from concourse import tile
from concourse.bass_utils import run_bass_kernel_spmd

F32 = mybir.dt.float32
BF16 = mybir.dt.bfloat16
FP8 = mybir.dt.float8e4
NP_BF16 = mybir.dt.np(BF16)
NP_FP8 = mybir.dt.np(FP8)

N_CORES = 8
B_FULL = 24576
L_IN = 900
L_PAD = 1024
B_LOC = B_FULL // N_CORES          # 3072
N_SUB = 512                        # samples per matmul moving tile
T_SUB = 2                          # sub-tiles per pass
N_PASS = 3
NCH_X = 8

KER, STR, PAD_L, PAD_R = 40, 2, 38, 39
N_LAYERS = 7

L1, L2 = 469, 254
NCH_P2 = 2

SRC_CHW = {
    "x":  [128] * 7 + [4],
    "P2": [128, 126],
}
FEAT_SCALE = [1.0 / 469, 1.0 / 254, 1.0 / 146, 1.0 / 92,
              1.0 / 65, 1.0 / 52, 1.0 / 45, 1.0 / 32]


# ----------------------------------------------------------------- host math
def _conv_map(M, w):
    Mp = np.pad(M, ((PAD_L, PAD_R), (0, 0)))
    Lo = (Mp.shape[0] - KER) // STR + 1
    out = np.zeros((Lo, M.shape[1]), dtype=M.dtype)
    for k in range(KER):
        out += w[k] * Mp[k : k + STR * Lo : STR, :]
    return out


def _build_groups(feat_w, pass_w):
    """Device row-groups.  Each group dict:
      src   'x' | 'P2'
      M     [R<=128, C] fp64 rows
      secs  list of (kind, off, rows, meta):
              ('f', row_off, n, (zt, za_off, [(fid, n), ...]))
              ('P', row_off, n, (dst_chunk, dst_off))
    Group order is the per-pass PE issue order.
    """
    fw = feat_w.reshape(N_LAYERS, KER).astype(np.float64)
    pw = pass_w.reshape(N_LAYERS, KER).astype(np.float64)

    I900 = np.eye(L_IN)
    F1 = _conv_map(I900, fw[0])          # [469, 900]
    P1 = _conv_map(I900, pw[0])
    F2 = _conv_map(P1, fw[1])            # [254, 900]  (composed: acts on x)
    P2 = _conv_map(P1, pw[1])            # [254, 900]
    I254 = np.eye(L2)
    F3 = _conv_map(I254, fw[2])
    P3 = _conv_map(I254, pw[2])
    F4 = _conv_map(P3, fw[3])
    P4 = _conv_map(P3, pw[3])
    F5 = _conv_map(P4, fw[4])
    P5 = _conv_map(P4, pw[4])
    F6 = _conv_map(P5, fw[5])
    P6 = _conv_map(P5, pw[5])
    F7 = _conv_map(P6, fw[6])
    PF = _conv_map(P6, pw[6])

    groups = []

    def f1g(a, b, zt):
        n = b - a
        groups.append(dict(src="x", M=F1[a:b], order=None,
                           secs=[("f", 0, n, (zt, 0, [(0, n)]))]))

    def s2g(a, b, zt, za_off, pchunk, poff):
        n = b - a
        # P-half sits at row offset 64 (engine partition bases must be
        # 32-aligned); any gap rows are zero weights -> zero psum rows
        M = np.concatenate([F2[a:b], np.zeros((64 - n, L_IN)), P2[a:b]])
        groups.append(dict(src="x", M=M, order=None,
                           secs=[("f", 0, n, (zt, za_off, [(1, n)])),
                                 ("P", 64, n, (pchunk, poff))]))

    # stage B rows in PSUM order (narrow-support first)
    s3rows = []
    s3rows += [(2, F3[r]) for r in range(64)]
    s3rows += [(3, F4[r]) for r in range(32)]
    s3rows += [(2, F3[r]) for r in range(64, 146)]
    s3rows += [(3, F4[r]) for r in range(32, 92)]
    s3rows += [(4, F5[r]) for r in range(65)]
    s3rows += [(5, F6[r]) for r in range(52)]
    s3rows += [(6, F7[r]) for r in range(45)]
    s3rows += [(7, PF[r]) for r in range(45)]
    assert len(s3rows) == 445
    s3_bounds = [0, 96, 224, 352, 445]

    def s3g(a, b, zt):
        blk = s3rows[a:b]
        M = np.stack([v for _, v in blk])
        feats = []
        for fid, _ in blk:
            if feats and feats[-1][0] == fid:
                feats[-1][1] += 1
            else:
                feats.append([fid, 1])
        groups.append(dict(src="P2", M=M, order=None,
                           secs=[("f", 0, b - a,
                                  (zt, 0, [(f, n) for f, n in feats]))]))

    # issue order: interleave f1/s2 by chunk availability; s3 last with a
    # 2-group gap after the final P2 copy
    f1g(0, 128, 0)
    s2g(0, 64, 4, 0, 0, 0)
    f1g(128, 256, 1)
    s2g(64, 128, 4, 64, 0, 64)
    s2g(128, 192, 5, 0, 1, 0)
    s2g(192, 254, 5, 64, 1, 64)
    f1g(256, 384, 2)
    f1g(384, 469, 3)
    for i, (a, b) in enumerate(zip(s3_bounds[:-1], s3_bounds[1:])):
        s3g(a, b, 6 + i)
    return groups


# zt tile remap: fp8 tiles 0-5 reordered so the DoubleRow pairs (0,1) (2,3)
# (4,5) have matching/compatible row counts; tiles 6-9 stay bf16
ZT_MAP = {0: 0, 1: 1, 2: 2, 4: 3, 3: 4, 5: 5, 6: 6, 7: 7, 8: 8, 9: 9}
N_ZT8 = 6          # fp8 za tiles (features 1-2: large pools, fp8-safe)
N_ZTB = 4          # bf16 za tiles (features 3-8)


def _pack_operands(groups):
    sched = []
    wts = []
    n_zt = 10
    sel = np.zeros((n_zt, 128, 8), dtype=np.float64)
    zt_rows = [0] * n_zt
    for g in groups:
        src, M = g["src"], g["M"]
        mrows = M.shape[0]
        chw = SRC_CHW[src]
        chunks = []
        for c in range(len(chw)):
            sub = M[:, c * 128 : c * 128 + chw[c]]
            if not np.any(sub != 0.0):
                continue
            chunks.append((len(wts), c))
            wts.append((sub.T, chw[c], mrows))
        secs = []
        for kind, off, n, meta in g["secs"]:
            if kind == "f":
                zt, za_off, feats = meta
                zt = ZT_MAP[zt]
                r = za_off
                for fid, cnt in feats:
                    sel[zt, r : r + cnt, fid] = 1.0
                    r += cnt
                zt_rows[zt] = max(zt_rows[zt], za_off + n)
                secs.append((kind, off, n, (zt, za_off, feats)))
            else:
                secs.append((kind, off, n, meta))
        sched.append(dict(src=src, chunks=chunks, mrows=mrows, secs=secs))

    n_wt = len(wts)
    wt = np.zeros((n_wt, 128, 128), dtype=np.float32)
    for i, (m, kw, mrows) in enumerate(wts):
        wt[i, :kw, :mrows] = m
    fscale = np.array(FEAT_SCALE, dtype=np.float32).reshape(8, 1)
    wt = np.ascontiguousarray(wt.transpose(1, 0, 2)).astype(NP_BF16)
    sel_f32 = sel.astype(np.float32).transpose(1, 0, 2)  # [128, zt, 8]
    sel8 = np.zeros((128, N_ZT8, 16), dtype=np.float32)
    sel8[:, :, :8] = sel_f32[:, :N_ZT8, :]
    sel8 = sel8.astype(NP_FP8)
    selb = np.ascontiguousarray(sel_f32[:, N_ZT8:, :]).astype(NP_BF16)
    return wt, sel8, selb, fscale, sched, zt_rows


# ------------------------------------------------- ldweights dedup (post-fin)
def _dedup_ldweights(nc):
    removed = 0
    for blk in nc.main_func.blocks:
        cur = None
        keep = []
        for inst in blk.instructions:
            if isinstance(inst, mybir.InstLdweights):
                sig = (repr(inst.ins[0]), inst.tile_position,
                       inst.perf_mode, inst.is_transpose)
                si = inst.sync_info
                clean = si is None or (len(si.on_wait) == 0
                                       and len(si.on_update) == 0)
                if clean and cur == sig:
                    removed += 1
                    continue
                cur = sig
                keep.append(inst)
            else:
                if isinstance(inst, mybir.InstMatmult):
                    if inst.is_transpose or inst.ldweights is not False:
                        cur = None
                keep.append(inst)
        del blk.instructions[:]
        blk.instructions.extend(keep)
    return removed


# ------------------------------------------------------------ device program
def _build_program(sched, n_wt, zt_rows):
    n_zt = len(zt_rows)
    nc = bacc.Bacc()
    xs_d = nc.dram_tensor("xs", [L_PAD, B_LOC], BF16, kind="ExternalInput")
    wt_d = nc.dram_tensor("wt", [128, n_wt, 128], BF16, kind="ExternalInput")
    sel8_d = nc.dram_tensor("sel8", [128, N_ZT8, 16], FP8,
                            kind="ExternalInput")
    selb_d = nc.dram_tensor("selb", [128, N_ZTB, 8], BF16, kind="ExternalInput")
    out_d = nc.dram_tensor("out", [2, 8, B_LOC], F32,
                           kind="ExternalOutput")

    TT = T_SUB * N_SUB                  # 1536
    QW = B_LOC // 4                     # x DMA piece width (768)
    with tile.TileContext(nc) as tc:
        with (
            tc.tile_pool(name="const", bufs=1) as constp,
            tc.tile_pool(name="ckpt", bufs=1) as ckptp,
            tc.tile_pool(name="za", bufs=2) as zapool,
        ):
            # DMA priority order: weights first (every ldweights needs its
            # slice), then the h=0 half of every x chunk (pass 1 reads only
            # cols [0:TT)), then the h=1 halves.  Two queues (sync, gpsimd)
            # share the load; per-chunk tiles keep deps range-granular.
            sel8_sb = constp.tile([128, N_ZT8, 16], FP8)
            nc.gpsimd.dma_start(sel8_sb[:], sel8_d[:])
            selb_sb = constp.tile([128, N_ZTB, 8], BF16)
            nc.gpsimd.dma_start(selb_sb[:], selb_d[:])
            wt_sb = constp.tile([128, n_wt, 128], BF16)
            wq = (n_wt + 3) // 4
            for w0 in range(0, n_wt, wq):
                w1 = min(n_wt, w0 + wq)
                nc.gpsimd.dma_start(wt_sb[:, w0:w1, :], wt_d[:, w0:w1, :])

            HW_ = B_LOC // 2
            x_cs = [constp.tile([128, B_LOC], BF16, name=f"x_c{c}")
                    for c in range(NCH_X)]
            # chunks 0-1 h0 split into quarters: the very first matmul only
            # needs cols [0:512) of chunk 0, so its wait clears ~1.5us sooner
            # first pieces fan out over three queues: sync/vector/scalar
            # engines are all idle at boot, so chunk0[0:512] and
            # chunk1[0:512] (the first matmul's reads) transfer in parallel
            qeng = [nc.sync, nc.scalar]
            for i, (c, q0, q1) in enumerate(
                    ((0, 0, 512), (1, 0, 512), (0, 512, 1024),
                     (1, 512, 1024), (0, 1024, 1536), (1, 1024, 1536))):
                qeng[i % 2].dma_start(
                    x_cs[c][:, q0:q1],
                    xs_d[c * 128 : (c + 1) * 128, q0:q1])
            for h in range(2):
                for c in range(NCH_X):
                    if h == 0 and c < 2:
                        continue
                    nc.sync.dma_start(
                        x_cs[c][:, h * HW_ : (h + 1) * HW_],
                        xs_d[c * 128 : (c + 1) * 128, h * HW_ : (h + 1) * HW_])

            p2_cs = [ckptp.tile([128, B_LOC], BF16, name=f"p2_c{c}")
                     for c in range(NCH_P2)]

            # split point for drain work: scalar takes [0:DS), vector the rest
            # (scalar: 0.833ns/el Abs; vector: 1.042 cast + 0.26 sign-mask)
            DS = 640

            with (
                tc.tile_pool(name="pg", bufs=3,
                             space=bass.MemorySpace.PSUM) as pgp,
                tc.tile_pool(name="pf", bufs=1,
                             space=bass.MemorySpace.PSUM) as pfp,
            ):
                for p in range(N_PASS):
                    s0 = p * TT
                    za8 = zapool.tile([128, N_ZT8, TT], FP8, tag="za8")
                    zab = zapool.tile([128, N_ZTB, TT], BF16, tag="zab")
                    # fp8 pair (4,5) contracts K=126 but tile 4 only has 85
                    # real rows: zero the tail so the PE never reads garbage
                    nc.vector.memset(za8[64:128, 4, :], 0.0)
                    for ent in sched:
                        src, mrows = ent["src"], ent["mrows"]
                        src_cs = x_cs if src == "x" else p2_cs
                        pg = pgp.tile([128, T_SUB, N_SUB], F32, tag="pg")
                        nj = len(ent["chunks"])
                        for j, (i, c) in enumerate(ent["chunks"]):
                            kw = SRC_CHW[src][c]
                            for t in range(T_SUB):
                                nc.tensor.matmul(
                                    pg[0:mrows, t, :],
                                    wt_sb[0:kw, i, 0:mrows],
                                    src_cs[c][0:kw,
                                              s0 + t * N_SUB : s0 + (t + 1) * N_SUB],
                                    start=(j == 0), stop=(j == nj - 1),
                                    skip_group_check=True)
                        for kind, off, n, meta in ent["secs"]:
                            view = pg[off : off + n].rearrange(
                                "p t n -> p (t n)")
                            if kind == "f":
                                zt, za_off, _ = meta
                                if zt < N_ZT8:
                                    dst = za8[za_off : za_off + n, zt, :]
                                    idt, msk = mybir.dt.uint8, 0x7F
                                else:
                                    dst = zab[za_off : za_off + n,
                                              zt - N_ZT8, :]
                                    idt, msk = mybir.dt.uint16, 0x7FFF
                                nc.scalar.activation(
                                    dst[:, 0:DS], view[:, 0:DS],
                                    mybir.ActivationFunctionType.Abs)
                                nc.vector.tensor_copy(
                                    dst[:, DS:TT], view[:, DS:TT])
                                du = dst[:, DS:TT].bitcast(idt)
                                nc.vector.tensor_scalar(
                                    du, du, msk, None,
                                    op0=mybir.AluOpType.bitwise_and)
                            else:
                                dc, doff = meta
                                dst = p2_cs[dc][doff : doff + n,
                                               s0 : s0 + TT]
                                nc.scalar.copy(dst[:, 0:DS], view[:, 0:DS])
                                nc.vector.tensor_copy(
                                    dst[:, DS:TT], view[:, DS:TT])

                    # selector burst + output for this pass: accumulate
                    # [8, N] feature sums in PSUM, copy to SBUF, DMA out
                    # (scale + transpose happen on the host)
                    # fp8 DoubleRow pairs and bf16 singles must NOT share
                    # one PSUM accumulation group (mode switch corrupts it):
                    # separate tiles, combined on the vector engine
                    HN = N_SUB // 2
                    NP8 = N_ZT8 // 2
                    for t in range(T_SUB):
                        pf8 = pfp.tile([16, N_SUB], F32, tag="pf8")
                        pfb = pfp.tile([8, N_SUB], F32, tag="pfb")
                        for h in range(2):
                            for pi in range(NP8):
                                kr = max(zt_rows[2 * pi], zt_rows[2 * pi + 1])
                                c0 = t * N_SUB + h * HN
                                nc.tensor.matmul(
                                    pf8[0:16, h * HN : (h + 1) * HN],
                                    sel8_sb[0:kr, 2 * pi : 2 * pi + 2, :],
                                    za8[0:kr, 2 * pi : 2 * pi + 2,
                                        c0 : c0 + HN],
                                    start=(pi == 0), stop=(pi == NP8 - 1),
                                    perf_mode=mybir.MatmulPerfMode.DoubleRow,
                                    skip_group_check=True)
                        for j in range(N_ZTB):
                            kr = zt_rows[N_ZT8 + j]
                            nc.tensor.matmul(
                                pfb[:],
                                selb_sb[0:kr, j, :],
                                zab[0:kr, j, t * N_SUB : (t + 1) * N_SUB],
                                start=(j == 0), stop=(j == N_ZTB - 1),
                                skip_group_check=True)
                        trow = s0 + t * N_SUB
                        fc8 = zapool.tile([8, N_SUB], F32, tag="fc8")
                        nc.vector.tensor_copy(fc8[:], pf8[0:8, :])
                        fcb = zapool.tile([8, N_SUB], F32, tag="fcb")
                        nc.scalar.copy(fcb[:], pfb[:])
                        oeng = nc.gpsimd if t % 2 == 0 else nc.sync
                        oeng.dma_start(
                            out_d[0, :, trow : trow + N_SUB], fc8[:])
                        oeng.dma_start(
                            out_d[1, :, trow : trow + N_SUB], fcb[:])
    nc.finalize()
    _dedup_ldweights(nc)
    return nc


_CACHE = {}


def _get_program(feat_w, pass_w):
    groups = _build_groups(feat_w, pass_w)
    wt, sel8, selb, fscale, sched, zt_rows = _pack_operands(groups)
    key = tuple((e["src"], e["mrows"], tuple(e["chunks"]), repr(e["secs"]))
                for e in sched)
    if key not in _CACHE:
        _CACHE[key] = _build_program(sched, wt.shape[1], zt_rows)
    return _CACHE[key], wt, sel8, selb, fscale


def _prepare(inputs):
    nc, wt, sel8, selb, fscale = _get_program(
        inputs["feat_w"], inputs["pass_w"])
    xsT = np.zeros((L_PAD, B_FULL), dtype=NP_BF16)
    xsT[:L_IN, :] = np.asarray(
        inputs["x"], dtype=np.float32).reshape(B_FULL, L_IN).T
    in_maps = [
        {"xs": np.ascontiguousarray(xsT[:, i * B_LOC : (i + 1) * B_LOC]),
         "wt": wt, "sel8": sel8, "selb": selb}
        for i in range(N_CORES)
    ]

    def post(res):
        out = np.concatenate(
            [res.results[i]["out"].sum(axis=0) for i in range(N_CORES)],
            axis=1)
        return np.ascontiguousarray((out * fscale).T.astype(np.float32))

    return nc, in_maps, post


def kernel(x, feat_w, pass_w):
    nc, in_maps, post = _prepare(
        {"x": x, "feat_w": feat_w, "pass_w": pass_w})
    res = run_bass_kernel_spmd(nc, in_maps, list(range(N_CORES)))
    return post(res)


if __name__ == "__main__":
    rng = np.random.default_rng(0)
    feat_w = (rng.standard_normal((7, 1, 1, 40)) * 0.1).astype(np.float32)
    pass_w = (rng.standard_normal((7, 1, 1, 40)) * 0.1).astype(np.float32)
    groups = _build_groups(feat_w, pass_w)
    wt, sel8, selb, fscale, sched, zt_rows = _pack_operands(groups)
    sel = np.concatenate([sel8.astype(np.float32),
                          selb.astype(np.float32)], axis=1)
    n_mm = sum(len(e["chunks"]) for e in sched)
    print(f"groups={len(sched)} mms/subtile={n_mm} zt_rows={zt_rows}")
    for e in sched:
        print(f"  {e['src']:3s} rows={e['mrows']:3d} "
              f"chunks={[c for _, c in e['chunks']]} "
              f"secs={[(k, o, n) for k, o, n, _ in e['secs']]}")

    def bf(a):
        return np.asarray(a, dtype=np.float32).astype(NP_BF16).astype(np.float64)

    B = 256
    x = rng.standard_normal((B, 1, L_IN)).astype(np.float32)
    xs = np.zeros((L_PAD, B))
    xs[:L_IN] = bf(x.reshape(B, L_IN).T)
    srcs = {"x": xs, "P2": np.zeros((NCH_P2 * 128, B))}
    za = np.zeros((10, 128, B))
    wtf = np.ascontiguousarray(wt.transpose(1, 0, 2)).astype(np.float64)
    for e in sched:
        acc = np.zeros((128, B))
        S = srcs[e["src"]]
        for i, c in e["chunks"]:
            kw = SRC_CHW[e["src"]][c]
            acc[: e["mrows"]] += (
                wtf[i, :kw, : e["mrows"]].T @ S[c * 128 : c * 128 + kw])
        for kind, off, n, meta in e["secs"]:
            if kind == "f":
                zt, za_off, _ = meta
                q = np.abs(acc[off : off + n]).astype(np.float32)
                if zt < N_ZT8:
                    q = q.astype(NP_FP8).astype(np.float64)
                else:
                    q = bf(q)
                za[zt, za_off : za_off + n] = q
            else:
                dc, doff = meta
                srcs["P2"][dc * 128 + doff : dc * 128 + doff + n] = bf(
                    acc[off : off + n])
    self_sel = np.ascontiguousarray(sel.transpose(1, 0, 2)).astype(np.float64)
    pf = np.zeros((8, B))
    for zt in range(10):
        kr = zt_rows[zt]
        pf += self_sel[zt, :kr].T @ za[zt, :kr]
    feats = (pf * fscale.astype(np.float64)).T

    fw = feat_w.reshape(7, 40).astype(np.float64)
    pw = pass_w.reshape(7, 40).astype(np.float64)
    px = x.reshape(B, L_IN).astype(np.float64)
    ref = []
    cur = px
    for i in range(7):
        xp = np.pad(cur, ((0, 0), (PAD_L, PAD_R)))
        Lo = (xp.shape[1] - KER) // STR + 1
        f = np.zeros((B, Lo)); nxt = np.zeros((B, Lo))
        for k in range(KER):
            sl = xp[:, k : k + STR * Lo : STR]
            f += fw[i, k] * sl
            nxt += pw[i, k] * sl
        ref.append(np.abs(f).mean(1))
        cur = nxt
    ref.append(np.abs(cur).sum(1) / 32.0)
    ref = np.stack(ref, 1)
    rel = np.abs(feats - ref) / np.maximum(np.abs(ref), 1e-6)
    print(f"host-emulated rel err vs fp64 reference: {rel.max():.3e}")

